# revision 32
# baseline (speedup 1.0000x reference)
"""MoE (63 routed experts top-7 + 1 shared expert) Trainium2 kernel.

Strategy: expert-parallel sparse dispatch. The router (softmax + top-k,
~0.3% of FLOPs) runs on host; tokens are gathered expert-major into
per-expert pieces, which are distributed across 8 NeuronCores. Each core
runs an identical (SPMD) Bass program with 9 slots:

- 8 routed slots in fp8-e4m3 with DoubleRow matmuls (2 fp8 weights per
  PE cell -> 2x MACs/cycle). Routed outputs are damped by gates
  (sum of top-7 gates <= ~0.54), so fp8 error stays ~4e-3 relative.
  Weights are pre-scaled x64 on host (w std 0.02 would sit in e4m3's
  subnormal range); the activation's pre-scale 1/64 dequantizes.
  Pieces are size-sorted and grouped across cores so each slot position
  is compiled with capacity = max piece size in its group (saves ~10%
  of the fp8 work vs a uniform 1024-token capacity).
- 1 fp16 shared-expert slot (ungated output needs the accuracy);
  8 cores x 1024 tokens covers all 8192 tokens exactly.

Feature-major layout (features on partitions, tokens on the free dim)
so weights need no transpose. Activations are staged partition-major in
DRAM so slot loads are contiguous per-partition runs. Outputs are
gathered and gate-weighted back on host in the reference's exact
accumulation order.
"""

import os
import sys
import math

sys.path.insert(0, "/opt/trn_rl_repo")

import numpy as np

D = 1280          # model dim
I = 1280          # expert inter dim
EXPERTS = 63      # routed experts
TOPK = 7          # routed top-k
CAP = 1024        # max tokens per weight slot
CHUNK = 512       # max tokens per matmul
CPS = CAP // CHUNK
KT = D // 128     # 10 contraction tiles of 128
NCORES = 8
WSCALE = 64.0     # fp8 weight pre-scale (dequantized in activation)

_PROGRAM_CACHE = {}


# ----------------------------------------------------------------- router

def _route(x2d, wr, br):
    """f32 softmax + top-k, matching jax.nn.softmax / jax.lax.top_k."""
    logits = (x2d @ wr + br).astype(np.float32)
    logits -= logits.max(-1, keepdims=True)
    np.exp(logits, out=logits)
    aff = logits / logits.sum(-1, keepdims=True)
    idx = np.argsort(-aff, axis=-1, kind="stable")[:, :TOPK]
    vals = np.take_along_axis(aff, idx, axis=-1)
    return idx.astype(np.int32), vals.astype(np.float32)


def _build_plan(T, idx):
    """Pack (token, expert) pairs expert-major into <=CAP-token routed
    pieces. Returns pieces, expert-major order, token-of-pair map."""
    flat = idx.ravel()
    order = np.argsort(flat, kind="stable")          # expert-major slot order
    tok_of = (order // TOPK).astype(np.int64)
    counts = np.bincount(flat, minlength=EXPERTS)
    offs = np.concatenate([[0], np.cumsum(counts)])

    pieces = []  # (expert, a, b)  [a:b) into the expert-major order
    for e in range(EXPERTS):
        a, b = int(offs[e]), int(offs[e + 1])
        while a < b:
            n = min(CAP, b - a)
            pieces.append((e, a, a + n))
            a += n
    return pieces, order, tok_of


def _chunks_of(cap):
    """Split a slot capacity into matmul chunk widths (<=CHUNK each)."""
    out = []
    off = 0
    while off < cap:
        w = min(CHUNK, cap - off)
        out.append((off, w))
        off += w
    return out


# ----------------------------------------------------------- device program

def _build_program(caps, zb):
    import concourse.bass as bass
    import concourse.mybir as mybir
    import concourse.tile as tile
    from concourse import bacc

    f32 = mybir.dt.float32
    f16 = mybir.dt.float16
    f8 = mybir.dt.float8e4
    S8 = len(caps)
    offs = np.concatenate([[0], np.cumsum(caps)]).astype(int)
    M8 = int(offs[-1])
    M = M8 + CAP  # + one shared slot

    nc = bacc.Bacc("TRN2", target_bir_lowering=False, debug=False,
                   enable_asserts=False, num_devices=NCORES)
    mult, add = mybir.AluOpType.mult, mybir.AluOpType.add
    # x staged partition-major: contiguous per-partition runs per k-pair
    x8T = nc.dram_tensor("x8T", [128, S8, CPS, KT, CHUNK], f8,
                         kind="ExternalInput").ap()
    x16T = nc.dram_tensor("x16T", [128, CPS, KT, CHUNK], f16,
                          kind="ExternalInput").ap()
    w1s = nc.dram_tensor("w1s", [S8, KT, 128, KT, 128], f8, kind="ExternalInput").ap()
    w2s = nc.dram_tensor("w2s", [S8, KT, 128, KT, 128], f8, kind="ExternalInput").ap()
    if not zb:
        b1s = nc.dram_tensor("b1s", [S8, 128, KT], f32, kind="ExternalInput").ap()
        b2s = nc.dram_tensor("b2s", [S8, 128, KT], f32, kind="ExternalInput").ap()
    sw1 = nc.dram_tensor("sw1", [KT, 128, KT, 128], f16, kind="ExternalInput").ap()
    sw2 = nc.dram_tensor("sw2", [KT, 128, KT, 128], f16, kind="ExternalInput").ap()
    if not zb:
        sb1 = nc.dram_tensor("sb1", [128, KT], f32, kind="ExternalInput").ap()
        sb2 = nc.dram_tensor("sb2", [128, KT], f32, kind="ExternalInput").ap()
    yT = nc.dram_tensor("yT", [KT, 128, M], f16, kind="ExternalOutput").ap()

    Gelu = mybir.ActivationFunctionType.Gelu
    Ident = mybir.ActivationFunctionType.Identity
    DR = mybir.MatmulPerfMode.DoubleRow
    DQ = 1.0 / WSCALE

    with tile.TileContext(nc) as tc:
        with (
            tc.tile_pool(name="xa", bufs=3) as xa,
            tc.tile_pool(name="w1p", bufs=6) as w1p,
            tc.tile_pool(name="w2p", bufs=6) as w2p,
            tc.tile_pool(name="hp", bufs=4) as hp,
            tc.tile_pool(name="yo", bufs=12) as yo,
            tc.tile_pool(name="bp", bufs=2) as bp,
            tc.tile_pool(name="xa6", bufs=2) as xa6,
            tc.tile_pool(name="w1p6", bufs=4) as w1p6,
            tc.tile_pool(name="w2p6", bufs=4) as w2p6,
            tc.tile_pool(name="hp6", bufs=2) as hp6,
            tc.tile_pool(name="ps", bufs=8, space="PSUM") as ps,
        ):
            # ---- routed fp8 DoubleRow slots ---------------------------
            for s in range(S8):
                cap = caps[s]
                chunks = _chunks_of(cap)
                col0 = int(offs[s])
                if not zb:
                    b1t = bp.tile([128, KT], f32, tag="b1", name="b1t")
                    nc.sync.dma_start(out=b1t[:, :], in_=b1s[s])
                    b2t = bp.tile([128, KT], f32, tag="b2", name="b2t")
                    nc.sync.dma_start(out=b2t[:, :], in_=b2s[s])

                # SWDGE loads at k-pair granularity: contiguous per-partition
                # runs, and the first DoubleRow matmul only waits for one pair
                xs = xa.tile([128, CPS, KT, CHUNK], f8, tag="x", name="xs")
                for c, (_, ccap) in enumerate(chunks):
                    for k in range(0, KT, 2):
                        if s == 0 and c == 0 and k == 0:
                            # first pair on HWDGE, issued before the first
                            # weight tile: the opening matmul's inputs land
                            # in ~2us instead of waiting out SWDGE startup
                            nc.sync.dma_start(out=xs[:, c, k:k + 2, :ccap],
                                              in_=x8T[:, s, c, k:k + 2, :ccap])
                        else:
                            nc.gpsimd.dma_start(out=xs[:, c, k:k + 2, :ccap],
                                                in_=x8T[:, s, c, k:k + 2, :ccap])

                hc = [hp.tile([128, KT, CHUNK], f8, tag=f"h{c}", name="hct")
                      for c in range(len(chunks))]

                # layer 1: h = gelu((x @ w1s)/64 [+ b1])
                for io in range(KT):
                    w1t = w1p.tile([128, KT, 128], f8, tag="w1", name="w1t")
                    nc.sync.dma_start(out=w1t[:, :, :], in_=w1s[s, io])
                    pts = [ps.tile([128, ccap], f32, tag=f"ps{c}", bufs=4,
                                   name="pt")
                           for c, (_, ccap) in enumerate(chunks)]
                    for k in range(0, KT, 2):
                        for c, (_, ccap) in enumerate(chunks):
                            nc.tensor.matmul(pts[c][:, :], w1t[:, k:k + 2, :],
                                             xs[:, c, k:k + 2, :ccap],
                                             start=(k == 0), stop=(k == KT - 2),
                                             perf_mode=DR)
                    for c, (_, ccap) in enumerate(chunks):
                        if zb:
                            nc.scalar.activation(hc[c][:, io, :ccap],
                                                 pts[c][:, :], Gelu, scale=DQ)
                        else:
                            nc.scalar.activation(hc[c][:, io, :ccap],
                                                 pts[c][:, :], Gelu,
                                                 bias=b1t[:, io:io + 1],
                                                 scale=DQ)

                # layer 2: y = (h @ w2s)/64 [+ b2]
                for io in range(KT):
                    w2t = w2p.tile([128, KT, 128], f8, tag="w2", name="w2t")
                    nc.sync.dma_start(out=w2t[:, :, :], in_=w2s[s, io])
                    pts = [ps.tile([128, ccap], f32, tag=f"ps{c}", bufs=4,
                                   name="pt")
                           for c, (_, ccap) in enumerate(chunks)]
                    # k descending: the group opens on the LAST-written h
                    # tiles (io 8,9), so it never holds a PSUM bank open
                    # waiting at the layer-1 boundary
                    for k in range(KT - 2, -2, -2):
                        for c, (_, ccap) in enumerate(chunks):
                            nc.tensor.matmul(pts[c][:, :], w2t[:, k:k + 2, :],
                                             hc[c][:, k:k + 2, :ccap],
                                             start=(k == KT - 2), stop=(k == 0),
                                             perf_mode=DR)
                    for c, (c_off, ccap) in enumerate(chunks):
                        yt = yo.tile([128, CHUNK], f16, tag="y", name="yt")
                        # DVE: y = psum*(1/64) [+ b2]  (keeps ScalarE free
                        # for the Gelus, which only it can run)
                        if zb:
                            nc.vector.tensor_scalar_mul(yt[:, :ccap],
                                                        pts[c][:, :], DQ)
                        else:
                            nc.scalar.activation(yt[:, :ccap], pts[c][:, :],
                                                 Ident,
                                                 bias=b2t[:, io:io + 1],
                                                 scale=DQ)
                        # output DMA rides ScalarE's queue, decoupled from
                        # the weight loads on Sync
                        nc.scalar.dma_start(
                            out=yT[io, :, col0 + c_off: col0 + c_off + ccap],
                            in_=yt[:, :ccap])

            # ---- 1 shared fp16 slot ----------------------------------
            col0 = M8
            if not zb:
                sb1t = bp.tile([128, KT], f32, tag="b1", name="sb1t")
                nc.sync.dma_start(out=sb1t[:, :], in_=sb1)
                sb2t = bp.tile([128, KT], f32, tag="b2", name="sb2t")
                nc.sync.dma_start(out=sb2t[:, :], in_=sb2)

            xs6 = xa6.tile([128, CPS, KT, CHUNK], f16, tag="x", name="xs6")
            for c in range(CPS):
                for k in range(0, KT, 2):
                    nc.gpsimd.dma_start(out=xs6[:, c, k:k + 2],
                                        in_=x16T[:, c, k:k + 2])
            xc = [xs6[:, c] for c in range(CPS)]

            hc = [hp6.tile([128, KT, CHUNK], f16, tag="h", name=f"h6{c}")
                  for c in range(CPS)]

            for io in range(KT):
                w1t = w1p6.tile([128, KT, 128], f16, tag="w1", name="w1t6")
                nc.sync.dma_start(out=w1t[:, :, :], in_=sw1[io])
                for c in range(CPS):
                    pt = ps.tile([128, CHUNK], f32, tag="ps0", bufs=4, name="pt")
                    for k in range(KT):
                        nc.tensor.matmul(pt[:, :], w1t[:, k, :], xc[c][:, k, :],
                                         start=(k == 0), stop=(k == KT - 1))
                    if zb:
                        nc.scalar.activation(hc[c][:, io, :], pt[:, :], Gelu)
                    else:
                        nc.scalar.activation(hc[c][:, io, :], pt[:, :], Gelu,
                                             bias=sb1t[:, io:io + 1])

            for io in range(KT):
                w2t = w2p6.tile([128, KT, 128], f16, tag="w2", name="w2t6")
                nc.sync.dma_start(out=w2t[:, :, :], in_=sw2[io])
                for c in range(CPS):
                    pt = ps.tile([128, CHUNK], f32, tag="ps0", bufs=4, name="pt")
                    for k in range(KT - 1, -1, -1):
                        nc.tensor.matmul(pt[:, :], w2t[:, k, :], hc[c][:, k, :],
                                         start=(k == KT - 1), stop=(k == 0))
                    yt = yo.tile([128, CHUNK], f16, tag="y", name="yt")
                    if zb:
                        nc.vector.tensor_scalar_mul(yt[:, :], pt[:, :], 1.0)
                    else:
                        nc.scalar.activation(yt[:, :], pt[:, :], Ident,
                                             bias=sb2t[:, io:io + 1])
                    nc.scalar.dma_start(
                        out=yT[io, :, col0 + c * CHUNK: col0 + (c + 1) * CHUNK],
                        in_=yt[:, :])
    nc.compile()
    return nc


def _get_program(caps, zb):
    key = (caps, zb)
    if key not in _PROGRAM_CACHE:
        _PROGRAM_CACHE[key] = _build_program(caps, zb)
    return _PROGRAM_CACHE[key]


# ------------------------------------------------------------------ kernel

def _f8():
    import ml_dtypes
    return ml_dtypes.float8_e4m3   # TRN FP8_EXP4: max 240, IEEE-style


def _arrange_w(w):
    """[D, I] -> [io, p, ko, c] so each (slot, io) block DMAs contiguously
    into an SBUF tile laid out [partition, ko, col]."""
    return np.ascontiguousarray(
        w.reshape(KT, 128, KT, 128).transpose(2, 1, 0, 3))


def kernel(x, sw1, sb1, sw2, sb2, rw1, rb1, rw2, rb2, wr, br, _trace=False):
    from concourse.bass_utils import run_bass_kernel_spmd

    f8dt = _f8()
    x = np.asarray(x, dtype=np.float32)
    B, Sq, _ = x.shape
    T = B * Sq
    xf = np.ascontiguousarray(x.reshape(T, D))

    idx, vals = _route(xf, np.asarray(wr, np.float32), np.asarray(br, np.float32))
    pieces, order, tok_of = _build_plan(T, idx)
    R = len(pieces)
    S8 = math.ceil(R / NCORES)
    assert T == NCORES * CAP, "shared slot layout assumes 8192 tokens"

    # size-sorted grouping: slot position j holds the pieces ranked
    # [8j, 8j+8) by size, one per core; its compiled capacity is the
    # group max (rounded up to 16)
    sizes = np.array([b - a for (_, a, b) in pieces])
    rank = np.argsort(-sizes, kind="stable")
    assign = {}           # (core, j) -> piece index
    caps = []
    for j in range(S8):
        grp = rank[j * NCORES: (j + 1) * NCORES]
        for core, p in enumerate(grp):
            assign[(core, j)] = int(p)
        caps.append(min(CAP, max(16, -(-int(sizes[grp].max()) // 16) * 16)))
    caps = tuple(caps)
    offs = np.concatenate([[0], np.cumsum(caps)]).astype(int)
    M8 = int(offs[-1])

    rw1 = np.asarray(rw1, np.float32); rw2 = np.asarray(rw2, np.float32)
    rb1 = np.asarray(rb1, np.float32); rb2 = np.asarray(rb2, np.float32)
    sw1 = np.asarray(sw1, np.float32); sw2 = np.asarray(sw2, np.float32)
    sb1 = np.asarray(sb1, np.float32); sb2 = np.asarray(sb2, np.float32)
    zb = not (rb1.any() or rb2.any() or sb1.any() or sb2.any())

    w1a = [_arrange_w(rw1[e] * WSCALE).astype(f8dt) for e in range(EXPERTS)]
    w2a = [_arrange_w(rw2[e] * WSCALE).astype(f8dt) for e in range(EXPERTS)]
    b1a = [np.ascontiguousarray(rb1[e].reshape(KT, 128).T) for e in range(EXPERTS)]
    b2a = [np.ascontiguousarray(rb2[e].reshape(KT, 128).T) for e in range(EXPERTS)]
    sw1a = _arrange_w(sw1).astype(np.float16)
    sw2a = _arrange_w(sw2).astype(np.float16)
    sb1a = np.ascontiguousarray(sb1.reshape(KT, 128).T)
    sb2a = np.ascontiguousarray(sb2.reshape(KT, 128).T)

    xfT = np.ascontiguousarray(xf.T)          # [D, T] f32
    xfT8 = xfT.astype(f8dt)
    xfT16 = xfT.astype(np.float16)

    in_maps = []
    for core in range(NCORES):
        x8_core = np.zeros((D, S8 * CAP), dtype=f8dt)
        w1_core = np.zeros((S8, KT, 128, KT, 128), dtype=f8dt)
        w2_core = np.zeros((S8, KT, 128, KT, 128), dtype=f8dt)
        b1_core = np.zeros((S8, 128, KT), dtype=np.float32)
        b2_core = np.zeros((S8, 128, KT), dtype=np.float32)
        for j in range(S8):
            p = assign.get((core, j))
            if p is None or p >= R:
                continue  # dummy slot: zero weights -> zero output
            e, a, b = pieces[p]
            x8_core[:, j * CAP: j * CAP + (b - a)] = xfT8[:, tok_of[a:b]]
            w1_core[j] = w1a[e]; w2_core[j] = w2a[e]
            b1_core[j] = b1a[e]; b2_core[j] = b2a[e]
        x16_core = xfT16[:, core * CAP: (core + 1) * CAP]
        im = {
            "x8T": np.ascontiguousarray(
                x8_core.reshape(KT, 128, S8, CPS, CHUNK)
                .transpose(1, 2, 3, 0, 4)),
            "x16T": np.ascontiguousarray(
                x16_core.reshape(KT, 128, CPS, CHUNK).transpose(1, 2, 0, 3)),
            "w1s": w1_core, "w2s": w2_core,
            "sw1": sw1a, "sw2": sw2a,
        }
        if not zb:
            im.update({"b1s": b1_core, "b2s": b2_core,
                       "sb1": sb1a, "sb2": sb2a})
        in_maps.append(im)

    nc = _get_program(caps, zb)
    res = run_bass_kernel_spmd(nc, in_maps, core_ids=list(range(NCORES)),
                               trace=_trace)
    kernel.last_result = res

    TK = T * TOPK
    M = M8 + CAP
    gated = np.empty((TK, D), dtype=np.float32)   # expert-major rows
    shared_out = np.empty((T, D), dtype=np.float32)
    for core in range(NCORES):
        Y = res.results[core]["yT"].astype(np.float32).reshape(D, M)
        for j in range(S8):
            p = assign.get((core, j))
            if p is None or p >= R:
                continue
            e, a, b = pieces[p]
            gated[a:b] = Y[:, int(offs[j]): int(offs[j]) + (b - a)].T
        shared_out[core * CAP: (core + 1) * CAP] = Y[:, M8:].T

    g = vals.ravel()[order].astype(np.float32)
    gated *= g[:, None]
    ord2 = np.argsort(tok_of, kind="stable")      # token-major, expert asc
    routed = gated[ord2].reshape(T, TOPK, D).sum(axis=1, dtype=np.float32)

    out = shared_out + routed + xf
    return out.reshape(B, Sq, D).astype(np.float32)


kernel.last_result = None


# revision 33
# speedup vs baseline: 1.0042x; 1.0042x over previous
"""MoE (63 routed experts top-7 + 1 shared expert) Trainium2 kernel.

Strategy: expert-parallel sparse dispatch. The router (softmax + top-k,
~0.3% of FLOPs) runs on host; tokens are gathered expert-major into
per-expert pieces, which are distributed across 8 NeuronCores. Each core
runs an identical (SPMD) Bass program with 9 slots:

- 8 routed slots in fp8-e4m3 with DoubleRow matmuls (2 fp8 weights per
  PE cell -> 2x MACs/cycle). Routed outputs are damped by gates
  (sum of top-7 gates <= ~0.54), so fp8 error stays ~4e-3 relative.
  Weights are pre-scaled x64 on host (w std 0.02 would sit in e4m3's
  subnormal range); the activation's pre-scale 1/64 dequantizes.
  Pieces are size-sorted and grouped across cores so each slot position
  is compiled with capacity = max piece size in its group (saves ~10%
  of the fp8 work vs a uniform 1024-token capacity).
- 1 fp16 shared-expert slot (ungated output needs the accuracy);
  8 cores x 1024 tokens covers all 8192 tokens exactly.

Feature-major layout (features on partitions, tokens on the free dim)
so weights need no transpose. Activations are staged partition-major in
DRAM so slot loads are contiguous per-partition runs. Outputs are
gathered and gate-weighted back on host in the reference's exact
accumulation order.
"""

import os
import sys
import math

sys.path.insert(0, "/opt/trn_rl_repo")

import numpy as np

D = 1280          # model dim
I = 1280          # expert inter dim
EXPERTS = 63      # routed experts
TOPK = 7          # routed top-k
CAP = 1024        # max tokens per weight slot
CHUNK = 512       # max tokens per matmul
CPS = CAP // CHUNK
KT = D // 128     # 10 contraction tiles of 128
NCORES = 8
WSCALE = 64.0     # fp8 weight pre-scale (dequantized in activation)

_PROGRAM_CACHE = {}


# ----------------------------------------------------------------- router

def _route(x2d, wr, br):
    """f32 softmax + top-k, matching jax.nn.softmax / jax.lax.top_k."""
    logits = (x2d @ wr + br).astype(np.float32)
    logits -= logits.max(-1, keepdims=True)
    np.exp(logits, out=logits)
    aff = logits / logits.sum(-1, keepdims=True)
    idx = np.argsort(-aff, axis=-1, kind="stable")[:, :TOPK]
    vals = np.take_along_axis(aff, idx, axis=-1)
    return idx.astype(np.int32), vals.astype(np.float32)


def _build_plan(T, idx):
    """Pack (token, expert) pairs expert-major into <=CAP-token routed
    pieces. Returns pieces, expert-major order, token-of-pair map."""
    flat = idx.ravel()
    order = np.argsort(flat, kind="stable")          # expert-major slot order
    tok_of = (order // TOPK).astype(np.int64)
    counts = np.bincount(flat, minlength=EXPERTS)
    offs = np.concatenate([[0], np.cumsum(counts)])

    pieces = []  # (expert, a, b)  [a:b) into the expert-major order
    for e in range(EXPERTS):
        a, b = int(offs[e]), int(offs[e + 1])
        while a < b:
            n = min(CAP, b - a)
            pieces.append((e, a, a + n))
            a += n
    return pieces, order, tok_of


def _chunks_of(cap):
    """Split a slot capacity into matmul chunk widths (<=CHUNK each)."""
    out = []
    off = 0
    while off < cap:
        w = min(CHUNK, cap - off)
        out.append((off, w))
        off += w
    return out


# ----------------------------------------------------------- device program

def _build_program(caps, zb):
    import concourse.bass as bass
    import concourse.mybir as mybir
    import concourse.tile as tile
    from concourse import bacc

    f32 = mybir.dt.float32
    f16 = mybir.dt.float16
    f8 = mybir.dt.float8e4
    S8 = len(caps)
    offs = np.concatenate([[0], np.cumsum(caps)]).astype(int)
    M8 = int(offs[-1])
    M = M8 + CAP  # + one shared slot

    nc = bacc.Bacc("TRN2", target_bir_lowering=False, debug=False,
                   enable_asserts=False, num_devices=NCORES)
    mult, add = mybir.AluOpType.mult, mybir.AluOpType.add
    # x staged partition-major: contiguous per-partition runs per k-pair
    x8T = nc.dram_tensor("x8T", [128, S8, CPS, KT, CHUNK], f8,
                         kind="ExternalInput").ap()
    x16T = nc.dram_tensor("x16T", [128, CPS, KT, CHUNK], f16,
                          kind="ExternalInput").ap()
    w1s = nc.dram_tensor("w1s", [S8, KT, 128, KT, 128], f8, kind="ExternalInput").ap()
    w2s = nc.dram_tensor("w2s", [S8, KT, 128, KT, 128], f8, kind="ExternalInput").ap()
    if not zb:
        b1s = nc.dram_tensor("b1s", [S8, 128, KT], f32, kind="ExternalInput").ap()
        b2s = nc.dram_tensor("b2s", [S8, 128, KT], f32, kind="ExternalInput").ap()
    sw1 = nc.dram_tensor("sw1", [KT, 128, KT, 128], f16, kind="ExternalInput").ap()
    sw2 = nc.dram_tensor("sw2", [KT, 128, KT, 128], f16, kind="ExternalInput").ap()
    if not zb:
        sb1 = nc.dram_tensor("sb1", [128, KT], f32, kind="ExternalInput").ap()
        sb2 = nc.dram_tensor("sb2", [128, KT], f32, kind="ExternalInput").ap()
    yT = nc.dram_tensor("yT", [KT, 128, M], f16, kind="ExternalOutput").ap()

    Gelu = mybir.ActivationFunctionType.Gelu
    Ident = mybir.ActivationFunctionType.Identity
    DR = mybir.MatmulPerfMode.DoubleRow
    DQ = 1.0 / WSCALE

    with tile.TileContext(nc) as tc:
        with (
            tc.tile_pool(name="xa", bufs=3) as xa,
            tc.tile_pool(name="w1p", bufs=6) as w1p,
            tc.tile_pool(name="w2p", bufs=6) as w2p,
            tc.tile_pool(name="hp", bufs=4) as hp,
            tc.tile_pool(name="yo", bufs=12) as yo,
            tc.tile_pool(name="bp", bufs=2) as bp,
            tc.tile_pool(name="xa6", bufs=2) as xa6,
            tc.tile_pool(name="w1p6", bufs=4) as w1p6,
            tc.tile_pool(name="w2p6", bufs=4) as w2p6,
            tc.tile_pool(name="hp6", bufs=2) as hp6,
            tc.tile_pool(name="ps", bufs=8, space="PSUM") as ps,
        ):
            # ---- routed fp8 DoubleRow slots ---------------------------
            for s in range(S8):
                cap = caps[s]
                chunks = _chunks_of(cap)
                col0 = int(offs[s])
                if not zb:
                    b1t = bp.tile([128, KT], f32, tag="b1", name="b1t")
                    nc.sync.dma_start(out=b1t[:, :], in_=b1s[s])
                    b2t = bp.tile([128, KT], f32, tag="b2", name="b2t")
                    nc.sync.dma_start(out=b2t[:, :], in_=b2s[s])

                # SWDGE loads at k-pair granularity: contiguous per-partition
                # runs, and the first DoubleRow matmul only waits for one pair
                xs = xa.tile([128, CPS, KT, CHUNK], f8, tag="x", name="xs")
                for c, (_, ccap) in enumerate(chunks):
                    for k in range(0, KT, 2):
                        if s == 0 and c == 0 and k == 0:
                            # first pair on HWDGE, issued before the first
                            # weight tile: the opening matmul's inputs land
                            # in ~2us instead of waiting out SWDGE startup
                            nc.sync.dma_start(out=xs[:, c, k:k + 2, :ccap],
                                              in_=x8T[:, s, c, k:k + 2, :ccap])
                        else:
                            nc.gpsimd.dma_start(out=xs[:, c, k:k + 2, :ccap],
                                                in_=x8T[:, s, c, k:k + 2, :ccap])

                hc = [hp.tile([128, KT, CHUNK], f8, tag=f"h{c}", name="hct")
                      for c in range(len(chunks))]

                # layer 1: h = gelu((x @ w1s)/64 [+ b1])
                for io in range(KT):
                    w1t = w1p.tile([128, KT, 128], f8, tag="w1", name="w1t")
                    nc.sync.dma_start(out=w1t[:, :, :], in_=w1s[s, io])
                    pts = [ps.tile([128, ccap], f32, tag=f"ps{c}", bufs=4,
                                   name="pt")
                           for c, (_, ccap) in enumerate(chunks)]
                    for k in range(0, KT, 2):
                        for c, (_, ccap) in enumerate(chunks):
                            nc.tensor.matmul(pts[c][:, :], w1t[:, k:k + 2, :],
                                             xs[:, c, k:k + 2, :ccap],
                                             start=(k == 0), stop=(k == KT - 2),
                                             perf_mode=DR)
                    for c, (_, ccap) in enumerate(chunks):
                        if zb:
                            nc.scalar.activation(hc[c][:, io, :ccap],
                                                 pts[c][:, :], Gelu, scale=DQ)
                        else:
                            nc.scalar.activation(hc[c][:, io, :ccap],
                                                 pts[c][:, :], Gelu,
                                                 bias=b1t[:, io:io + 1],
                                                 scale=DQ)

                # layer 2: y = (h @ w2s)/64 [+ b2]
                for io in range(KT):
                    w2t = w2p.tile([128, KT, 128], f8, tag="w2", name="w2t")
                    nc.sync.dma_start(out=w2t[:, :, :], in_=w2s[s, io])
                    pts = [ps.tile([128, ccap], f32, tag=f"ps{c}", bufs=4,
                                   name="pt")
                           for c, (_, ccap) in enumerate(chunks)]
                    # k descending: the group opens on the LAST-written h
                    # tiles (io 8,9), so it never holds a PSUM bank open
                    # waiting at the layer-1 boundary
                    for k in range(KT - 2, -2, -2):
                        for c, (_, ccap) in enumerate(chunks):
                            nc.tensor.matmul(pts[c][:, :], w2t[:, k:k + 2, :],
                                             hc[c][:, k:k + 2, :ccap],
                                             start=(k == KT - 2), stop=(k == 0),
                                             perf_mode=DR)
                    for c, (c_off, ccap) in enumerate(chunks):
                        yt = yo.tile([128, CHUNK], f16, tag="y", name="yt")
                        # DVE: y = psum*(1/64) [+ b2]  (keeps ScalarE free
                        # for the Gelus, which only it can run)
                        if zb:
                            nc.vector.tensor_scalar_mul(yt[:, :ccap],
                                                        pts[c][:, :], DQ)
                        else:
                            nc.scalar.activation(yt[:, :ccap], pts[c][:, :],
                                                 Ident,
                                                 bias=b2t[:, io:io + 1],
                                                 scale=DQ)
                        # output DMA rides ScalarE's queue, decoupled from
                        # the weight loads on Sync
                        nc.scalar.dma_start(
                            out=yT[io, :, col0 + c_off: col0 + c_off + ccap],
                            in_=yt[:, :ccap])

            # ---- 1 shared fp16 slot ----------------------------------
            col0 = M8
            if not zb:
                sb1t = bp.tile([128, KT], f32, tag="b1", name="sb1t")
                nc.sync.dma_start(out=sb1t[:, :], in_=sb1)
                sb2t = bp.tile([128, KT], f32, tag="b2", name="sb2t")
                nc.sync.dma_start(out=sb2t[:, :], in_=sb2)

            xs6 = xa6.tile([128, CPS, KT, CHUNK], f16, tag="x", name="xs6")
            for c in range(CPS):
                for k in range(0, KT, 2):
                    nc.gpsimd.dma_start(out=xs6[:, c, k:k + 2],
                                        in_=x16T[:, c, k:k + 2])
            xc = [xs6[:, c] for c in range(CPS)]

            hc = [hp6.tile([128, KT, CHUNK], f16, tag="h", name=f"h6{c}")
                  for c in range(CPS)]

            for io in range(KT):
                w1t = w1p6.tile([128, KT, 128], f16, tag="w1", name="w1t6")
                nc.sync.dma_start(out=w1t[:, :, :], in_=sw1[io])
                for c in range(CPS):
                    pt = ps.tile([128, CHUNK], f32, tag="ps0", bufs=4, name="pt")
                    for k in range(KT):
                        nc.tensor.matmul(pt[:, :], w1t[:, k, :], xc[c][:, k, :],
                                         start=(k == 0), stop=(k == KT - 1))
                    if zb:
                        nc.scalar.activation(hc[c][:, io, :], pt[:, :], Gelu)
                    else:
                        nc.scalar.activation(hc[c][:, io, :], pt[:, :], Gelu,
                                             bias=sb1t[:, io:io + 1])

            for io in range(KT):
                w2t = w2p6.tile([128, KT, 128], f16, tag="w2", name="w2t6")
                nc.sync.dma_start(out=w2t[:, :, :], in_=sw2[io])
                for c in range(CPS):
                    pt = ps.tile([128, CHUNK], f32, tag="ps0", bufs=4, name="pt")
                    for k in range(KT - 1, -1, -1):
                        nc.tensor.matmul(pt[:, :], w2t[:, k, :], hc[c][:, k, :],
                                         start=(k == KT - 1), stop=(k == 0))
                    yt = yo.tile([128, CHUNK], f16, tag="y", name="yt")
                    if zb:
                        nc.vector.tensor_scalar_mul(yt[:, :], pt[:, :], 1.0)
                    else:
                        nc.scalar.activation(yt[:, :], pt[:, :], Ident,
                                             bias=sb2t[:, io:io + 1])
                    nc.scalar.dma_start(
                        out=yT[io, :, col0 + c * CHUNK: col0 + (c + 1) * CHUNK],
                        in_=yt[:, :])
    nc.compile()
    return nc


def _get_program(caps, zb):
    key = (caps, zb)
    if key not in _PROGRAM_CACHE:
        _PROGRAM_CACHE[key] = _build_program(caps, zb)
    return _PROGRAM_CACHE[key]


# ------------------------------------------------------------------ kernel

def _f8():
    import ml_dtypes
    return ml_dtypes.float8_e4m3   # TRN FP8_EXP4: max 240, IEEE-style


def _arrange_w(w):
    """[D, I] -> [io, p, ko, c] so each (slot, io) block DMAs contiguously
    into an SBUF tile laid out [partition, ko, col]."""
    return np.ascontiguousarray(
        w.reshape(KT, 128, KT, 128).transpose(2, 1, 0, 3))


def kernel(x, sw1, sb1, sw2, sb2, rw1, rb1, rw2, rb2, wr, br, _trace=False):
    from concourse.bass_utils import run_bass_kernel_spmd

    f8dt = _f8()
    x = np.asarray(x, dtype=np.float32)
    B, Sq, _ = x.shape
    T = B * Sq
    xf = np.ascontiguousarray(x.reshape(T, D))

    idx, vals = _route(xf, np.asarray(wr, np.float32), np.asarray(br, np.float32))
    pieces, order, tok_of = _build_plan(T, idx)
    R = len(pieces)
    S8 = math.ceil(R / NCORES)
    assert T == NCORES * CAP, "shared slot layout assumes 8192 tokens"

    # size-sorted grouping: slot position j holds the pieces ranked
    # [8j, 8j+8) by size, one per core; its compiled capacity is the
    # group max (rounded up to 16)
    sizes = np.array([b - a for (_, a, b) in pieces])
    rank = np.argsort(-sizes, kind="stable")
    assign = {}           # (core, j) -> piece index
    caps = []
    for j in range(S8):
        grp = rank[j * NCORES: (j + 1) * NCORES]
        for core, p in enumerate(grp):
            assign[(core, j)] = int(p)
        caps.append(min(CAP, max(16, -(-int(sizes[grp].max()) // 8) * 8)))
    caps = tuple(caps)
    offs = np.concatenate([[0], np.cumsum(caps)]).astype(int)
    M8 = int(offs[-1])

    rw1 = np.asarray(rw1, np.float32); rw2 = np.asarray(rw2, np.float32)
    rb1 = np.asarray(rb1, np.float32); rb2 = np.asarray(rb2, np.float32)
    sw1 = np.asarray(sw1, np.float32); sw2 = np.asarray(sw2, np.float32)
    sb1 = np.asarray(sb1, np.float32); sb2 = np.asarray(sb2, np.float32)
    zb = not (rb1.any() or rb2.any() or sb1.any() or sb2.any())

    w1a = [_arrange_w(rw1[e] * WSCALE).astype(f8dt) for e in range(EXPERTS)]
    w2a = [_arrange_w(rw2[e] * WSCALE).astype(f8dt) for e in range(EXPERTS)]
    b1a = [np.ascontiguousarray(rb1[e].reshape(KT, 128).T) for e in range(EXPERTS)]
    b2a = [np.ascontiguousarray(rb2[e].reshape(KT, 128).T) for e in range(EXPERTS)]
    sw1a = _arrange_w(sw1).astype(np.float16)
    sw2a = _arrange_w(sw2).astype(np.float16)
    sb1a = np.ascontiguousarray(sb1.reshape(KT, 128).T)
    sb2a = np.ascontiguousarray(sb2.reshape(KT, 128).T)

    xfT = np.ascontiguousarray(xf.T)          # [D, T] f32
    xfT8 = xfT.astype(f8dt)
    xfT16 = xfT.astype(np.float16)

    in_maps = []
    for core in range(NCORES):
        x8_core = np.zeros((D, S8 * CAP), dtype=f8dt)
        w1_core = np.zeros((S8, KT, 128, KT, 128), dtype=f8dt)
        w2_core = np.zeros((S8, KT, 128, KT, 128), dtype=f8dt)
        b1_core = np.zeros((S8, 128, KT), dtype=np.float32)
        b2_core = np.zeros((S8, 128, KT), dtype=np.float32)
        for j in range(S8):
            p = assign.get((core, j))
            if p is None or p >= R:
                continue  # dummy slot: zero weights -> zero output
            e, a, b = pieces[p]
            x8_core[:, j * CAP: j * CAP + (b - a)] = xfT8[:, tok_of[a:b]]
            w1_core[j] = w1a[e]; w2_core[j] = w2a[e]
            b1_core[j] = b1a[e]; b2_core[j] = b2a[e]
        x16_core = xfT16[:, core * CAP: (core + 1) * CAP]
        im = {
            "x8T": np.ascontiguousarray(
                x8_core.reshape(KT, 128, S8, CPS, CHUNK)
                .transpose(1, 2, 3, 0, 4)),
            "x16T": np.ascontiguousarray(
                x16_core.reshape(KT, 128, CPS, CHUNK).transpose(1, 2, 0, 3)),
            "w1s": w1_core, "w2s": w2_core,
            "sw1": sw1a, "sw2": sw2a,
        }
        if not zb:
            im.update({"b1s": b1_core, "b2s": b2_core,
                       "sb1": sb1a, "sb2": sb2a})
        in_maps.append(im)

    nc = _get_program(caps, zb)
    res = run_bass_kernel_spmd(nc, in_maps, core_ids=list(range(NCORES)),
                               trace=_trace)
    kernel.last_result = res

    TK = T * TOPK
    M = M8 + CAP
    gated = np.empty((TK, D), dtype=np.float32)   # expert-major rows
    shared_out = np.empty((T, D), dtype=np.float32)
    for core in range(NCORES):
        Y = res.results[core]["yT"].astype(np.float32).reshape(D, M)
        for j in range(S8):
            p = assign.get((core, j))
            if p is None or p >= R:
                continue
            e, a, b = pieces[p]
            gated[a:b] = Y[:, int(offs[j]): int(offs[j]) + (b - a)].T
        shared_out[core * CAP: (core + 1) * CAP] = Y[:, M8:].T

    g = vals.ravel()[order].astype(np.float32)
    gated *= g[:, None]
    ord2 = np.argsort(tok_of, kind="stable")      # token-major, expert asc
    routed = gated[ord2].reshape(T, TOPK, D).sum(axis=1, dtype=np.float32)

    out = shared_out + routed + xf
    return out.reshape(B, Sq, D).astype(np.float32)


kernel.last_result = None
